# revision 7
# baseline (speedup 1.0000x reference)
"""BlockGRU Trainium2 kernel.

Strategy:
  - Data-parallel over batch: 64 sequences -> 8 cores x 8 sequences.
  - Phase A (bulk): x-side projections x_g = emb @ Wg_x + b_g for all t,
    computed as W-stationary GEMMs in "transposed layout" (hidden j on
    partitions, time on free dim). Embeddings are pre-transposed on host;
    results spill to a DRAM scratch buffer.
  - Phase B (recurrence): 2048 sequential GRU steps, everything in
    transposed layout: tiles (128 partitions = j', 16 free = (jc, b)).
    Recurrent matmuls W-stationary: lhsT = W tile (k x j), rhs = h (k x b).
    sigmoid/tanh on ScalarE (one table set), elementwise on VectorE.
  - Outputs stored transposed [j', t, jc, b]; host untransposes.

Layout mapping (per core): j = jc*128 + j'; tile[j', jc*8 + b] = value[b, j].
"""

import functools
import numpy as np

import concourse.bass as bass
import concourse.bacc as bacc
import concourse.mybir as mybir
import concourse.tile as tile
from concourse.bass_utils import run_bass_kernel_spmd

f32 = mybir.dt.float32
AF = mybir.ActivationFunctionType
ALU = mybir.AluOpType

INPUT = 256
HID = 256
B = 64
S = 2048
NCORES = 8
BL = B // NCORES  # 8 sequences per core


def build(seq=S, bl=BL, tw=128):
    """Build + compile the per-core SPMD program."""
    tw = min(tw, seq)
    assert seq % tw == 0
    tq = min(512, seq)  # bulk GEMM time-quarter (moving N)
    assert seq % tq == 0
    nc = bacc.Bacc("TRN2", target_bir_lowering=False, debug=False)

    # [ic, i', b, t] host-pre-transposed embeddings
    embt_d = nc.dram_tensor("embt", (2, 128, bl, seq), f32, kind="ExternalInput")
    h0_d = nc.dram_tensor("h0", (bl, HID), f32, kind="ExternalInput")
    #   wrec[g*4 + kc*2 + jc] = Wg_h[kc*128:+128, jc*128:+128], g in (z, r, h)
    #   wx  [g*4 + ic*2 + jc] = Wg_x[ic*128:+128, jc*128:+128]
    wrec_d = nc.dram_tensor("wrec", (12, 128, 128), f32, kind="ExternalInput")
    wx_d = nc.dram_tensor("wx", (12, 128, 128), f32, kind="ExternalInput")
    bias_d = nc.dram_tensor("bias", (3, 2, 128), f32, kind="ExternalInput")
    # transposed output [j', t, jc, b]; host untransposes
    out_d = nc.dram_tensor("out", (128, seq, 2, bl), f32, kind="ExternalOutput")
    # x-preact scratch: [gjb, j', t] with gjb = g*16 + jc*8 + b (g: z,r,h)
    xs_d = nc.dram_tensor("xs", (48, 128, seq), f32, kind="Internal")

    embt = embt_d.ap()
    out = out_d.ap()
    xs = xs_d.ap()

    with tile.TileContext(nc) as tc:
        with tc.tile_pool(name="const", bufs=1) as constp:
            wrec_sb = constp.tile([128, 12, 128], f32)
            nc.sync.dma_start(wrec_sb[:], wrec_d.ap().rearrange("i k j -> k i j"))
            wx_sb = constp.tile([128, 12, 128], f32)
            nc.sync.dma_start(wx_sb[:], wx_d.ap().rearrange("i k j -> k i j"))
            bias_sb = constp.tile([128, 6], f32)
            nc.sync.dma_start(
                bias_sb[:], bias_d.ap().rearrange("g jc jp -> jp (g jc)")
            )
            h0_sb = constp.tile([128, 16], f32)
            for jc in range(2):
                nc.sync.dma_start(
                    h0_sb[:, jc * 8 : (jc + 1) * 8],
                    h0_d.ap()[:, jc * 128 : (jc + 1) * 128].rearrange("b jp -> jp b"),
                )

            # ---------------- Phase A: bulk x-side projections ----------------
            with (
                tc.tile_pool(name="bulk_sb", bufs=2) as bsb,
                tc.tile_pool(name="bulk_ps", bufs=1, space="PSUM") as bps,
                tc.tile_pool(name="bulk_ev", bufs=4) as bev,
            ):
                for bi in range(bl):
                    embt_sb = []
                    for ic in range(2):
                        et = bsb.tile([128, seq], f32, tag=f"embt{ic}")
                        nc.sync.dma_start(et[:], embt[ic, :, bi, :])
                        embt_sb.append(et)
                    for g in range(3):
                        for jc in range(2):
                            for tqi in range(seq // tq):
                                ps = bps.tile([128, tq], f32, tag=f"ps{tqi % 4}")
                                for ic in range(2):
                                    nc.tensor.matmul(
                                        ps[:],
                                        wx_sb[:, g * 4 + ic * 2 + jc, :],
                                        embt_sb[ic][:, tqi * tq : (tqi + 1) * tq],
                                        start=(ic == 0),
                                        stop=(ic == 1),
                                    )
                                ev = bev.tile([128, tq], f32, tag=f"ev{tqi % 4}")
                                # evacuate + bias add; alternate ACT/DVE
                                bsl = bias_sb[:, g * 2 + jc : g * 2 + jc + 1]
                                if tqi % 2 == 0:
                                    nc.scalar.activation(
                                        ev[:], ps[:], AF.Identity, bias=bsl
                                    )
                                else:
                                    nc.vector.tensor_scalar_add(ev[:], ps[:], bsl)
                                nc.sync.dma_start(
                                    xs[
                                        g * 16 + jc * 8 + bi,
                                        :,
                                        tqi * tq : (tqi + 1) * tq,
                                    ],
                                    ev[:],
                                )

            # ---------------- Phase B: recurrence ----------------
            with (
                tc.tile_pool(name="xw", bufs=2) as xwp,
                tc.tile_pool(name="ow", bufs=2) as owp,
                tc.tile_pool(name="step_sb", bufs=3) as ssb,
                tc.tile_pool(name="step_ps", bufs=2, space="PSUM") as sps,
            ):
                hprev = h0_sb[:]
                for w in range(seq // tw):
                    xw = xwp.tile([128, 48, tw], f32, tag="xw")
                    nc.sync.dma_start(
                        xw[:],
                        xs[:, :, w * tw : (w + 1) * tw].rearrange("g p t -> p g t"),
                    )
                    ow = owp.tile([128, tw, 16], f32, tag="ow")
                    for t in range(tw):
                        pzr = sps.tile([128, 32], f32, tag="pzr")
                        for g in range(2):
                            for jc in range(2):
                                o = pzr[:, g * 16 + jc * 8 : g * 16 + jc * 8 + 8]
                                for kc in range(2):
                                    nc.tensor.matmul(
                                        o,
                                        wrec_sb[:, g * 4 + kc * 2 + jc, :],
                                        hprev[:, kc * 8 : kc * 8 + 8],
                                        start=(kc == 0),
                                        stop=(kc == 1),
                                    )
                        prez = ssb.tile([128, 32], f32, tag="prez")
                        nc.vector.tensor_add(prez[:], pzr[:], xw[:, 0:32, t])
                        zr = ssb.tile([128, 32], f32, tag="zr")
                        nc.scalar.activation(zr[:], prez[:], AF.Sigmoid)
                        rh = ssb.tile([128, 16], f32, tag="rh")
                        nc.vector.tensor_mul(rh[:], zr[:, 16:32], hprev)
                        ph = sps.tile([128, 16], f32, tag="ph")
                        for jc in range(2):
                            o = ph[:, jc * 8 : jc * 8 + 8]
                            for kc in range(2):
                                nc.tensor.matmul(
                                    o,
                                    wrec_sb[:, 8 + kc * 2 + jc, :],
                                    rh[:, kc * 8 : kc * 8 + 8],
                                    start=(kc == 0),
                                    stop=(kc == 1),
                                )
                        preh = ssb.tile([128, 16], f32, tag="preh")
                        nc.vector.tensor_add(preh[:], ph[:], xw[:, 32:48, t])
                        hti = ssb.tile([128, 16], f32, tag="hti")
                        nc.scalar.activation(hti[:], preh[:], AF.Tanh)
                        # A' = (z - 1) * h  (off critical path)
                        a_ = ssb.tile([128, 16], f32, tag="a_")
                        nc.vector.scalar_tensor_tensor(
                            a_[:], zr[:, 0:16], 1.0, hprev,
                            op0=ALU.subtract, op1=ALU.mult,
                        )
                        # B = z * htilde ; h' = B - A'
                        b_ = ssb.tile([128, 16], f32, tag="b_")
                        nc.vector.tensor_mul(b_[:], zr[:, 0:16], hti[:])
                        hnew = ow[:, t, :]
                        nc.vector.tensor_sub(hnew, b_[:], a_[:])
                        hprev = hnew
                    nc.sync.dma_start(out[:, w * tw : (w + 1) * tw, :, :], ow[:])

    nc.compile()
    return nc


@functools.lru_cache(maxsize=8)
def _built():
    return build()


def prep_weights(W_r, b_r, W_z, b_z, W_h, b_h):
    H = HID
    Wg = [np.asarray(w, np.float32) for w in (W_z, W_r, W_h)]  # gate order z, r, h
    bg = [np.asarray(b, np.float32) for b in (b_z, b_r, b_h)]
    wrec = np.ascontiguousarray(
        np.stack(
            [
                Wg[g][kc * 128 : (kc + 1) * 128, jc * 128 : (jc + 1) * 128]
                for g in range(3)
                for kc in range(2)
                for jc in range(2)
            ]
        )
    )
    wx = np.ascontiguousarray(
        np.stack(
            [
                Wg[g][H + ic * 128 : H + (ic + 1) * 128, jc * 128 : (jc + 1) * 128]
                for g in range(3)
                for ic in range(2)
                for jc in range(2)
            ]
        )
    )
    bias = np.ascontiguousarray(
        np.stack(
            [bg[g][jc * 128 : (jc + 1) * 128] for g in range(3) for jc in range(2)]
        ).reshape(3, 2, 128)
    )
    return wrec, wx, bias


def make_in_maps(embeddings, hidden, W_r, b_r, W_z, b_z, W_h, b_h, ncores=NCORES):
    wrec, wx, bias = prep_weights(W_r, b_r, W_z, b_z, W_h, b_h)
    emb = np.asarray(embeddings, np.float32)
    h0 = np.asarray(hidden, np.float32)
    bl = emb.shape[0] // ncores
    seq = emb.shape[1]
    # [ic, i', b, t]
    embt_all = np.ascontiguousarray(
        emb.reshape(emb.shape[0], seq, 2, 128).transpose(2, 3, 0, 1)
    )
    return [
        {
            "embt": np.ascontiguousarray(embt_all[:, :, c * bl : (c + 1) * bl, :]),
            "h0": np.ascontiguousarray(h0[c * bl : (c + 1) * bl]),
            "wrec": wrec,
            "wx": wx,
            "bias": bias,
        }
        for c in range(ncores)
    ]


def gather_out(res, ncores=NCORES):
    outs = []
    for c in range(ncores):
        ot = res.results[c]["out"]  # [j', t, jc, b]
        outs.append(np.ascontiguousarray(ot.transpose(3, 1, 2, 0)).reshape(
            ot.shape[3], ot.shape[1], HID))
    outputs = np.concatenate(outs, axis=0)
    return outputs


def kernel(embeddings, hidden, W_r, b_r, W_z, b_z, W_h, b_h):
    nc = _built()
    in_maps = make_in_maps(embeddings, hidden, W_r, b_r, W_z, b_z, W_h, b_h)
    res = run_bass_kernel_spmd(nc, in_maps, core_ids=list(range(NCORES)))
    outputs = gather_out(res)
    final_hidden = np.ascontiguousarray(outputs[:, -1, :])
    return outputs, final_hidden


# revision 11
# speedup vs baseline: 1.1234x; 1.1234x over previous
"""BlockGRU Trainium2 kernel.

Strategy:
  - Data-parallel over batch: 64 sequences -> 8 cores x 8 sequences.
  - Phase A (bulk): x-side projections x_g = emb @ Wg_x + b_g for all t,
    computed as W-stationary GEMMs in "transposed layout" (hidden j on
    partitions, time on free dim). Embeddings are pre-transposed on host;
    results spill to a DRAM scratch buffer.
  - Phase B (recurrence): 2048 sequential GRU steps, everything in
    transposed layout: tiles (128 partitions = j', 16 free = (jc, b)).
    Recurrent matmuls W-stationary: lhsT = W tile (k x j), rhs = h (k x b).
    sigmoid/tanh on ScalarE (one table set), elementwise on VectorE.
  - Outputs stored transposed [j', t, jc, b]; host untransposes.

Layout mapping (per core): j = jc*128 + j'; tile[j', jc*8 + b] = value[b, j].
"""

import functools
import numpy as np

import concourse.bass as bass
import concourse.bacc as bacc
import concourse.mybir as mybir
import concourse.tile as tile
from concourse.bass_utils import run_bass_kernel_spmd

f32 = mybir.dt.float32
AF = mybir.ActivationFunctionType
ALU = mybir.AluOpType

INPUT = 256
HID = 256
B = 64
S = 2048
NCORES = 8
BL = B // NCORES  # 8 sequences per core


def build(seq=S, bl=BL, tw=128):
    """Build + compile the per-core SPMD program."""
    tw = min(tw, seq)
    assert seq % tw == 0
    tq = min(512, seq)  # bulk GEMM time-quarter (moving N)
    assert seq % tq == 0
    nc = bacc.Bacc("TRN2", target_bir_lowering=False, debug=False)

    # [ic, i', b, t] host-pre-transposed embeddings
    embt_d = nc.dram_tensor("embt", (2, 128, bl, seq), f32, kind="ExternalInput")
    h0_d = nc.dram_tensor("h0", (bl, HID), f32, kind="ExternalInput")
    #   wrec[g*4 + kc*2 + jc] = Wg_h[kc*128:+128, jc*128:+128], g in (z, r, h)
    #   wx  [g*4 + ic*2 + jc] = Wg_x[ic*128:+128, jc*128:+128]
    wrec_d = nc.dram_tensor("wrec", (12, 128, 128), f32, kind="ExternalInput")
    wx_d = nc.dram_tensor("wx", (12, 128, 128), f32, kind="ExternalInput")
    bias_d = nc.dram_tensor("bias", (3, 2, 128), f32, kind="ExternalInput")
    ident_d = nc.dram_tensor("ident", (128, 128), f32, kind="ExternalInput")
    # transposed output [j', t, jc, b]; host untransposes
    out_d = nc.dram_tensor("out", (128, seq, 2, bl), f32, kind="ExternalOutput")
    # x-preact scratch: [gjb, j', t] with gjb = g*16 + jc*8 + b (g: z,r,h)
    xs_d = nc.dram_tensor("xs", (48, 128, seq), f32, kind="Internal")

    embt = embt_d.ap()
    out = out_d.ap()
    xs = xs_d.ap()

    with tile.TileContext(nc) as tc:
        with tc.tile_pool(name="const", bufs=1) as constp:
            wrec_sb = constp.tile([128, 12, 128], f32)
            nc.sync.dma_start(wrec_sb[:], wrec_d.ap().rearrange("i k j -> k i j"))
            wx_sb = constp.tile([128, 12, 128], f32)
            nc.sync.dma_start(wx_sb[:], wx_d.ap().rearrange("i k j -> k i j"))
            bias_sb = constp.tile([128, 6], f32)
            nc.sync.dma_start(
                bias_sb[:], bias_d.ap().rearrange("g jc jp -> jp (g jc)")
            )
            ident_sb = constp.tile([128, 128], f32)
            nc.sync.dma_start(ident_sb[:], ident_d.ap()[:])
            h0_sb = constp.tile([128, 16], f32)
            for jc in range(2):
                nc.sync.dma_start(
                    h0_sb[:, jc * 8 : (jc + 1) * 8],
                    h0_d.ap()[:, jc * 128 : (jc + 1) * 128].rearrange("b jp -> jp b"),
                )

            # ---------------- Phase A: bulk x-side projections ----------------
            with (
                tc.tile_pool(name="bulk_sb", bufs=2) as bsb,
                tc.tile_pool(name="bulk_ps", bufs=1, space="PSUM") as bps,
                tc.tile_pool(name="bulk_ev", bufs=4) as bev,
            ):
                for bi in range(bl):
                    embt_sb = []
                    for ic in range(2):
                        et = bsb.tile([128, seq], f32, tag=f"embt{ic}")
                        nc.sync.dma_start(et[:], embt[ic, :, bi, :])
                        embt_sb.append(et)
                    for g in range(3):
                        for jc in range(2):
                            for tqi in range(seq // tq):
                                ps = bps.tile([128, tq], f32, tag=f"ps{tqi % 4}")
                                for ic in range(2):
                                    nc.tensor.matmul(
                                        ps[:],
                                        wx_sb[:, g * 4 + ic * 2 + jc, :],
                                        embt_sb[ic][:, tqi * tq : (tqi + 1) * tq],
                                        start=(ic == 0),
                                        stop=(ic == 1),
                                    )
                                ev = bev.tile([128, tq], f32, tag=f"ev{tqi % 4}")
                                # evacuate + bias add; alternate ACT/DVE
                                bsl = bias_sb[:, g * 2 + jc : g * 2 + jc + 1]
                                if tqi % 2 == 0:
                                    nc.scalar.activation(
                                        ev[:], ps[:], AF.Identity, bias=bsl
                                    )
                                else:
                                    nc.vector.tensor_scalar_add(ev[:], ps[:], bsl)
                                nc.sync.dma_start(
                                    xs[
                                        g * 16 + jc * 8 + bi,
                                        :,
                                        tqi * tq : (tqi + 1) * tq,
                                    ],
                                    ev[:],
                                )

            # ---------------- Phase B: recurrence ----------------
            with (
                tc.tile_pool(name="xw", bufs=2) as xwp,
                tc.tile_pool(name="ow", bufs=2) as owp,
                tc.tile_pool(name="step_sb", bufs=3) as ssb,
                tc.tile_pool(name="step_ps", bufs=2, space="PSUM") as sps,
            ):
                hprev = h0_sb[:]
                for w in range(seq // tw):
                    xw = xwp.tile([128, 48, tw], f32, tag="xw")
                    nc.sync.dma_start(
                        xw[:],
                        xs[:, :, w * tw : (w + 1) * tw].rearrange("g p t -> p g t"),
                    )
                    ow = owp.tile([128, tw, 16], f32, tag="ow")
                    for t in range(tw):
                        # one PSUM bank holds all three gate preacts:
                        # [z(0:16) | r(16:32) | h(32:48)], seeded with the
                        # x-side preacts via a single identity matmul.
                        ps48 = sps.tile([128, 48], f32, tag="ps48")
                        nc.tensor.matmul(
                            ps48[:], ident_sb[:], xw[:, :, t],
                            start=True, stop=False, skip_group_check=True,
                        )
                        for g in range(2):
                            for jc in range(2):
                                o = ps48[:, g * 16 + jc * 8 : g * 16 + jc * 8 + 8]
                                for kc in range(2):
                                    nc.tensor.matmul(
                                        o,
                                        wrec_sb[:, g * 4 + kc * 2 + jc, :],
                                        hprev[:, kc * 8 : kc * 8 + 8],
                                        start=False,
                                        stop=(kc == 1),
                                        skip_group_check=True,
                                    )
                        zr = ssb.tile([128, 32], f32, tag="zr")
                        nc.scalar.activation(zr[:], ps48[:, 0:32], AF.Sigmoid)
                        rh = ssb.tile([128, 16], f32, tag="rh")
                        nc.vector.tensor_mul(rh[:], zr[:, 16:32], hprev)
                        for jc in range(2):
                            o = ps48[:, 32 + jc * 8 : 32 + jc * 8 + 8]
                            for kc in range(2):
                                nc.tensor.matmul(
                                    o,
                                    wrec_sb[:, 8 + kc * 2 + jc, :],
                                    rh[:, kc * 8 : kc * 8 + 8],
                                    start=False,
                                    stop=(kc == 1),
                                    skip_group_check=True,
                                )
                        hti = ssb.tile([128, 16], f32, tag="hti")
                        nc.scalar.activation(hti[:], ps48[:, 32:48], AF.Tanh)
                        # A' = (z - 1) * h  (off critical path)
                        a_ = ssb.tile([128, 16], f32, tag="a_")
                        nc.vector.scalar_tensor_tensor(
                            a_[:], zr[:, 0:16], 1.0, hprev,
                            op0=ALU.subtract, op1=ALU.mult,
                        )
                        # B = z * htilde ; h' = B - A'
                        b_ = ssb.tile([128, 16], f32, tag="b_")
                        nc.vector.tensor_mul(b_[:], zr[:, 0:16], hti[:])
                        hnew = ow[:, t, :]
                        nc.vector.tensor_sub(hnew, b_[:], a_[:])
                        hprev = hnew
                    nc.sync.dma_start(out[:, w * tw : (w + 1) * tw, :, :], ow[:])

    nc.compile()
    return nc


@functools.lru_cache(maxsize=8)
def _built():
    return build()


def prep_weights(W_r, b_r, W_z, b_z, W_h, b_h):
    H = HID
    Wg = [np.asarray(w, np.float32) for w in (W_z, W_r, W_h)]  # gate order z, r, h
    bg = [np.asarray(b, np.float32) for b in (b_z, b_r, b_h)]
    wrec = np.ascontiguousarray(
        np.stack(
            [
                Wg[g][kc * 128 : (kc + 1) * 128, jc * 128 : (jc + 1) * 128]
                for g in range(3)
                for kc in range(2)
                for jc in range(2)
            ]
        )
    )
    wx = np.ascontiguousarray(
        np.stack(
            [
                Wg[g][H + ic * 128 : H + (ic + 1) * 128, jc * 128 : (jc + 1) * 128]
                for g in range(3)
                for ic in range(2)
                for jc in range(2)
            ]
        )
    )
    bias = np.ascontiguousarray(
        np.stack(
            [bg[g][jc * 128 : (jc + 1) * 128] for g in range(3) for jc in range(2)]
        ).reshape(3, 2, 128)
    )
    return wrec, wx, bias


def make_in_maps(embeddings, hidden, W_r, b_r, W_z, b_z, W_h, b_h, ncores=NCORES):
    wrec, wx, bias = prep_weights(W_r, b_r, W_z, b_z, W_h, b_h)
    emb = np.asarray(embeddings, np.float32)
    h0 = np.asarray(hidden, np.float32)
    bl = emb.shape[0] // ncores
    seq = emb.shape[1]
    # [ic, i', b, t]
    embt_all = np.ascontiguousarray(
        emb.reshape(emb.shape[0], seq, 2, 128).transpose(2, 3, 0, 1)
    )
    ident = np.eye(128, dtype=np.float32)
    return [
        {
            "embt": np.ascontiguousarray(embt_all[:, :, c * bl : (c + 1) * bl, :]),
            "h0": np.ascontiguousarray(h0[c * bl : (c + 1) * bl]),
            "wrec": wrec,
            "wx": wx,
            "bias": bias,
            "ident": ident,
        }
        for c in range(ncores)
    ]


def gather_out(res, ncores=NCORES):
    outs = []
    for c in range(ncores):
        ot = res.results[c]["out"]  # [j', t, jc, b]
        outs.append(np.ascontiguousarray(ot.transpose(3, 1, 2, 0)).reshape(
            ot.shape[3], ot.shape[1], HID))
    outputs = np.concatenate(outs, axis=0)
    return outputs


def kernel(embeddings, hidden, W_r, b_r, W_z, b_z, W_h, b_h):
    nc = _built()
    in_maps = make_in_maps(embeddings, hidden, W_r, b_r, W_z, b_z, W_h, b_h)
    res = run_bass_kernel_spmd(nc, in_maps, core_ids=list(range(NCORES)))
    outputs = gather_out(res)
    final_hidden = np.ascontiguousarray(outputs[:, -1, :])
    return outputs, final_hidden


# revision 15
# speedup vs baseline: 1.3442x; 1.1966x over previous
"""BlockGRU Trainium2 kernel.

Strategy:
  - Data-parallel over batch: 64 sequences -> 8 cores x 8 sequences.
  - Phase A (bulk): x-side projections x_g = emb @ Wg_x + b_g for all t,
    computed as W-stationary GEMMs in "transposed layout" (hidden j on
    partitions, time on free dim). Embeddings are pre-transposed on host;
    results spill to a DRAM scratch buffer.
  - Phase B (recurrence): 2048 sequential GRU steps, everything in
    transposed layout: tiles (128 partitions = j', 16 free = (jc, b)).
    Recurrent matmuls W-stationary: lhsT = W tile (k x j), rhs = h (k x b).
    sigmoid/tanh on ScalarE (one table set), elementwise on VectorE.
  - Outputs stored transposed [j', t, jc, b]; host untransposes.

Layout mapping (per core): j = jc*128 + j'; tile[j', jc*8 + b] = value[b, j].
"""

import functools
import numpy as np

import concourse.bass as bass
import concourse.bacc as bacc
import concourse.mybir as mybir
import concourse.tile as tile
from concourse.bass_utils import run_bass_kernel_spmd

f32 = mybir.dt.float32
AF = mybir.ActivationFunctionType
ALU = mybir.AluOpType

INPUT = 256
HID = 256
B = 64
S = 2048
NCORES = 8
BL = B // NCORES  # 8 sequences per core


def build(seq=S, bl=BL, tw=128):
    """Build + compile the per-core SPMD program."""
    tw = min(tw, seq)
    assert seq % tw == 0
    tq = min(512, seq)  # bulk GEMM time-quarter (moving N)
    assert seq % tq == 0
    nc = bacc.Bacc("TRN2", target_bir_lowering=False, debug=False)

    # [ic, i', b, t] host-pre-transposed embeddings
    embt_d = nc.dram_tensor("embt", (2, 128, bl, seq), f32, kind="ExternalInput")
    h0_d = nc.dram_tensor("h0", (bl, HID), f32, kind="ExternalInput")
    #   wrec[g*4 + kc*2 + jc] = Wg_h[kc*128:+128, jc*128:+128], g in (z, r, h)
    #   wx  [g*4 + ic*2 + jc] = Wg_x[ic*128:+128, jc*128:+128]
    wrec_d = nc.dram_tensor("wrec", (12, 128, 128), f32, kind="ExternalInput")
    wx_d = nc.dram_tensor("wx", (12, 128, 128), f32, kind="ExternalInput")
    bias_d = nc.dram_tensor("bias", (3, 2, 128), f32, kind="ExternalInput")
    ident_d = nc.dram_tensor("ident", (128, 128), f32, kind="ExternalInput")
    # transposed output [j', t, jc, b]; host untransposes
    out_d = nc.dram_tensor("out", (128, seq, 2, bl), f32, kind="ExternalOutput")
    # x-preact scratch: [gjb, j', t] with gjb = g*16 + jc*8 + b (g: z,r,h)
    xs_d = nc.dram_tensor("xs", (48, 128, seq), f32, kind="Internal")

    embt = embt_d.ap()
    out = out_d.ap()
    xs = xs_d.ap()

    with tile.TileContext(nc) as tc:
        with tc.tile_pool(name="const", bufs=1) as constp:
            wrec_sb = constp.tile([128, 12, 128], f32)
            nc.sync.dma_start(wrec_sb[:], wrec_d.ap().rearrange("i k j -> k i j"))
            wx_sb = constp.tile([128, 12, 128], f32)
            nc.sync.dma_start(wx_sb[:], wx_d.ap().rearrange("i k j -> k i j"))
            bias_sb = constp.tile([128, 6], f32)
            nc.sync.dma_start(
                bias_sb[:], bias_d.ap().rearrange("g jc jp -> jp (g jc)")
            )
            ident_sb = constp.tile([128, 128], f32)
            nc.sync.dma_start(ident_sb[:], ident_d.ap()[:])
            h0_sb = constp.tile([128, 16], f32)
            for jc in range(2):
                nc.sync.dma_start(
                    h0_sb[:, jc * 8 : (jc + 1) * 8],
                    h0_d.ap()[:, jc * 128 : (jc + 1) * 128].rearrange("b jp -> jp b"),
                )

            # ---------------- Phase A: bulk x-side projections ----------------
            with (
                tc.tile_pool(name="bulk_sb", bufs=2) as bsb,
                tc.tile_pool(name="bulk_ps", bufs=1, space="PSUM") as bps,
                tc.tile_pool(name="bulk_ev", bufs=4) as bev,
            ):
                for bi in range(bl):
                    embt_sb = []
                    for ic in range(2):
                        et = bsb.tile([128, seq], f32, tag=f"embt{ic}")
                        nc.sync.dma_start(et[:], embt[ic, :, bi, :])
                        embt_sb.append(et)
                    for g in range(3):
                        for jc in range(2):
                            for tqi in range(seq // tq):
                                ps = bps.tile([128, tq], f32, tag=f"ps{tqi % 4}")
                                for ic in range(2):
                                    nc.tensor.matmul(
                                        ps[:],
                                        wx_sb[:, g * 4 + ic * 2 + jc, :],
                                        embt_sb[ic][:, tqi * tq : (tqi + 1) * tq],
                                        start=(ic == 0),
                                        stop=(ic == 1),
                                    )
                                ev = bev.tile([128, tq], f32, tag=f"ev{tqi % 4}")
                                # evacuate + bias add; alternate ACT/DVE
                                bsl = bias_sb[:, g * 2 + jc : g * 2 + jc + 1]
                                if tqi % 2 == 0:
                                    nc.scalar.activation(
                                        ev[:], ps[:], AF.Identity, bias=bsl
                                    )
                                else:
                                    nc.vector.tensor_scalar_add(ev[:], ps[:], bsl)
                                nc.sync.dma_start(
                                    xs[
                                        g * 16 + jc * 8 + bi,
                                        :,
                                        tqi * tq : (tqi + 1) * tq,
                                    ],
                                    ev[:],
                                )

            # ---------------- Phase B: recurrence ----------------
            with (
                tc.tile_pool(name="xw", bufs=2) as xwp,
                tc.tile_pool(name="ow", bufs=2) as owp,
                tc.tile_pool(name="step_sb", bufs=3) as ssb,
                tc.tile_pool(name="step_ps", bufs=2, space="PSUM") as sps,
            ):
                hprev = h0_sb[:]
                for w in range(seq // tw):
                    xw = xwp.tile([128, 48, tw], f32, tag="xw")
                    nc.sync.dma_start(
                        xw[:],
                        xs[:, :, w * tw : (w + 1) * tw].rearrange("g p t -> p g t"),
                    )
                    ow = owp.tile([128, tw, 16], f32, tag="ow")
                    for t in range(tw):
                        # one PSUM bank holds all three gate preacts:
                        # [z(0:16) | r(16:32) | h(32:48)], seeded with the
                        # x-side preacts via a single identity matmul.
                        ps48 = sps.tile([128, 48], f32, tag="ps48")
                        nc.tensor.matmul(
                            ps48[:], ident_sb[:], xw[:, :, t],
                            start=True, stop=False, skip_group_check=True,
                        )
                        for g in range(2):
                            for jc in range(2):
                                o = ps48[:, g * 16 + jc * 8 : g * 16 + jc * 8 + 8]
                                for kc in range(2):
                                    nc.tensor.matmul(
                                        o,
                                        wrec_sb[:, g * 4 + kc * 2 + jc, :],
                                        hprev[:, kc * 8 : kc * 8 + 8],
                                        start=False,
                                        stop=(kc == 1),
                                        skip_group_check=True,
                                    )
                        zr = ssb.tile([128, 32], f32, tag="zr")
                        nc.scalar.activation(zr[:], ps48[:, 0:32], AF.Sigmoid)
                        rh = ssb.tile([128, 16], f32, tag="rh")
                        nc.vector.tensor_mul(rh[:], zr[:, 16:32], hprev)
                        for jc in range(2):
                            o = ps48[:, 32 + jc * 8 : 32 + jc * 8 + 8]
                            for kc in range(2):
                                nc.tensor.matmul(
                                    o,
                                    wrec_sb[:, 8 + kc * 2 + jc, :],
                                    rh[:, kc * 8 : kc * 8 + 8],
                                    start=False,
                                    stop=(kc == 1),
                                    skip_group_check=True,
                                )
                        hti = ssb.tile([128, 16], f32, tag="hti")
                        nc.scalar.activation(hti[:], ps48[:, 32:48], AF.Tanh)
                        # A' = (z - 1) * h  (off critical path, on GpSimd)
                        a_ = ssb.tile([128, 16], f32, tag="a_")
                        nc.vector.scalar_tensor_tensor(
                            a_[:], zr[:, 0:16], 1.0, hprev,
                            op0=ALU.subtract, op1=ALU.mult,
                        )
                        # B = z * htilde ; h' = B - A'
                        b_ = ssb.tile([128, 16], f32, tag="b_")
                        nc.vector.tensor_mul(b_[:], zr[:, 0:16], hti[:])
                        hnew = ow[:, t, :]
                        nc.vector.tensor_sub(hnew, b_[:], a_[:])
                        hprev = hnew
                    nc.sync.dma_start(out[:, w * tw : (w + 1) * tw, :, :], ow[:])

    nc.compile()
    return nc


@functools.lru_cache(maxsize=8)
def _built():
    return build()


def prep_weights(W_r, b_r, W_z, b_z, W_h, b_h):
    H = HID
    Wg = [np.asarray(w, np.float32) for w in (W_z, W_r, W_h)]  # gate order z, r, h
    bg = [np.asarray(b, np.float32) for b in (b_z, b_r, b_h)]
    wrec = np.ascontiguousarray(
        np.stack(
            [
                Wg[g][kc * 128 : (kc + 1) * 128, jc * 128 : (jc + 1) * 128]
                for g in range(3)
                for kc in range(2)
                for jc in range(2)
            ]
        )
    )
    wx = np.ascontiguousarray(
        np.stack(
            [
                Wg[g][H + ic * 128 : H + (ic + 1) * 128, jc * 128 : (jc + 1) * 128]
                for g in range(3)
                for ic in range(2)
                for jc in range(2)
            ]
        )
    )
    bias = np.ascontiguousarray(
        np.stack(
            [bg[g][jc * 128 : (jc + 1) * 128] for g in range(3) for jc in range(2)]
        ).reshape(3, 2, 128)
    )
    return wrec, wx, bias


def make_in_maps(embeddings, hidden, W_r, b_r, W_z, b_z, W_h, b_h, ncores=NCORES):
    wrec, wx, bias = prep_weights(W_r, b_r, W_z, b_z, W_h, b_h)
    emb = np.asarray(embeddings, np.float32)
    h0 = np.asarray(hidden, np.float32)
    bl = emb.shape[0] // ncores
    seq = emb.shape[1]
    # [ic, i', b, t]
    embt_all = np.ascontiguousarray(
        emb.reshape(emb.shape[0], seq, 2, 128).transpose(2, 3, 0, 1)
    )
    ident = np.eye(128, dtype=np.float32)
    return [
        {
            "embt": np.ascontiguousarray(embt_all[:, :, c * bl : (c + 1) * bl, :]),
            "h0": np.ascontiguousarray(h0[c * bl : (c + 1) * bl]),
            "wrec": wrec,
            "wx": wx,
            "bias": bias,
            "ident": ident,
        }
        for c in range(ncores)
    ]


def gather_out(res, ncores=NCORES):
    outs = []
    for c in range(ncores):
        ot = res.results[c]["out"]  # [j', t, jc, b]
        outs.append(np.ascontiguousarray(ot.transpose(3, 1, 2, 0)).reshape(
            ot.shape[3], ot.shape[1], HID))
    outputs = np.concatenate(outs, axis=0)
    return outputs


def kernel(embeddings, hidden, W_r, b_r, W_z, b_z, W_h, b_h):
    nc = _built()
    in_maps = make_in_maps(embeddings, hidden, W_r, b_r, W_z, b_z, W_h, b_h)
    res = run_bass_kernel_spmd(nc, in_maps, core_ids=list(range(NCORES)))
    outputs = gather_out(res)
    final_hidden = np.ascontiguousarray(outputs[:, -1, :])
    return outputs, final_hidden
